# revision 2
# baseline (speedup 1.0000x reference)
"""ChildSum TreeLSTM cell on 8 Trainium2 NeuronCores (Bass/Tile, SPMD).

Sharding: nodes are re-packed host-side into 128 windows of exactly 128
nodes, balanced so every window has exactly 512 children (kmax=4 slots of
128, zero padding) whenever a balanced packing exists; otherwise windows are
padded to the max slot count.  16 windows per core.  Segment sums become
per-window one-hot matmuls accumulated in PSUM; one-hot masks are built on
device from a shipped rel-index tensor, so a single data-independent program
serves all 8 cores.  Outputs are inverse-permuted on the host.

All matmul operands are bf16 (PSUM accumulates fp32); biases are folded into
the matmuls as K=1 ones-row outer products.  DMAs are spread across the two
HWDGE queues (SP, ACT) plus the SWDGE (gpsimd) queue for output stores.

Per-core device program, per window w (software-pipelined one deep):
  s16t[k]  = one-hot S_cn (DVE iota compare), snc = S_cn.T (PE transpose)
  f_inp    = x_w @ Wwf.T + (bwf+buf)                (PSUM, bias matmul)
  fhg[k]   = prevh_k @ Wuf.T + snc_k @ f_inp        (one PSUM group)
  fjk[k]   = sigmoid(fhg[k])   (ACT) ; t[k] = fjk*prevc_k (DVE, bf16)
  fc       = sum_k S_cn_k.T-matmul(t[k])            (PSUM accum)
  htT      = sum_k prevh_k.T-matmul(S_cn_k)         (PSUM accum, q-outer)
  big(w-1) = [x;h_tilde] @ Wc.T + bc ; gates; c = sig(zi)*tanh(zu)+fc ;
  h        = sig(zo)*tanh(c)
"""

import numpy as np
import ml_dtypes

import concourse.bass as bass
import concourse.bacc as bacc
import concourse.mybir as mybir
from concourse import tile
from concourse.bass_utils import run_bass_kernel_spmd

BF16 = ml_dtypes.bfloat16
F32 = mybir.dt.float32
BF = mybir.dt.bfloat16

N, E, D, H = 16384, 65536, 512, 512
NCORES = 8
NL = N // NCORES            # 2048 local nodes
NW = NL // 128              # 16 windows per core
NWIN = NCORES * NW          # 128 windows total
H3 = 3 * H

AF = mybir.ActivationFunctionType
ALU = mybir.AluOpType

# ---------------------------------------------------------------------------
# Host-side planning: balanced node->window packing
# ---------------------------------------------------------------------------
def _plan(seg):
    """Assign nodes to NWIN windows of exactly 128 nodes, minimizing the max
    children per window.  Returns (order, kmax) where order is the node
    permutation (window-major: order[w*128:(w+1)*128] are window w's nodes)
    and kmax = max over windows of ceil(children/128)."""
    cnt = np.bincount(seg, minlength=N).astype(np.int64)
    target = E // NWIN  # 512

    # snake-deal nodes (sorted by child count desc) into windows
    srt = np.argsort(-cnt, kind="stable")
    rows = srt.reshape(N // NWIN, NWIN).copy()
    rows[1::2] = rows[1::2, ::-1]
    # windows[w] = list of node ids
    win = [list(rows[:, w]) for w in range(NWIN)]
    sums = np.array([int(cnt[w].sum()) for w in win], dtype=np.int64)

    # repair: swap nodes between over- and under-full windows until all
    # windows hit the target exactly (possible in practice for random trees).
    # Index nodes by count per window for O(1) swap lookup.
    from collections import defaultdict

    bycnt = [defaultdict(list) for _ in range(NWIN)]
    for w in range(NWIN):
        for nd in win[w]:
            bycnt[w][int(cnt[nd])].append(nd)

    def swap(wa, ca, wb, cb):
        a = bycnt[wa][ca].pop()
        b = bycnt[wb][cb].pop()
        bycnt[wa][cb].append(b)
        bycnt[wb][ca].append(a)
        sums[wa] += cb - ca
        sums[wb] += ca - cb

    for _ in range(200000):
        hi = int(np.argmax(sums))
        lo = int(np.argmin(sums))
        if sums[hi] == target and sums[lo] == target:
            break
        need = min(sums[hi] - target, target - sums[lo])
        done = False
        for d in range(max(int(need), 1), 0, -1):
            for ca, la in bycnt[hi].items():
                if not la:
                    continue
                cb = ca - d
                if bycnt[lo].get(cb):
                    swap(hi, ca, lo, cb)
                    done = True
                    break
            if done:
                break
        if not done:
            break  # no improving swap found

    order = np.empty(N, dtype=np.int64)
    for w in range(NWIN):
        nodes = [nd for la in bycnt[w].values() for nd in la]
        assert len(nodes) == 128
        order[w * 128 : (w + 1) * 128] = nodes
    kmax = int(np.max((sums + 127) // 128))
    return order, kmax


def _ragged(starts, counts):
    total = int(counts.sum())
    if total == 0:
        return np.empty(0, np.int64)
    out = np.ones(total, np.int64)
    nz = counts > 0
    s, c = starts[nz], counts[nz]
    out[0] = s[0]
    idx = np.cumsum(c)[:-1]
    out[idx] = s[1:] - (s[:-1] + c[:-1] - 1)
    return np.cumsum(out)


def _prep_core(inputs, core, order, kmax):
    seg = np.asarray(inputs["seg_ids"])
    x = np.asarray(inputs["x"])
    prev_c, prev_h = np.asarray(inputs["prev_c"]), np.asarray(inputs["prev_h"])
    cnt = np.bincount(seg, minlength=N).astype(np.int64)
    starts = np.concatenate([[0], np.cumsum(cnt)[:-1]])

    S = NW * kmax * 128
    nodes = order[core * NL : (core + 1) * NL]  # window-major node ids

    prevh_n = np.zeros((S, H), np.float32)
    prevc_n = np.zeros((S, H), np.float32)
    rel = np.full((S,), -1.0, np.float32)
    for w in range(NW):
        wn = nodes[w * 128 : (w + 1) * 128]
        idx = _ragged(starts[wn], cnt[wn])
        m = len(idx)
        base = w * kmax * 128
        assert m <= kmax * 128
        prevh_n[base : base + m] = prev_h[idx]
        prevc_n[base : base + m] = prev_c[idx]
        rel[base : base + m] = np.repeat(np.arange(128), cnt[wn]).astype(np.float32)[
            : m
        ]

    return {
        "x_T": np.ascontiguousarray(x[nodes].T).astype(BF16),            # [D, NL]
        "prevh_T": np.ascontiguousarray(prevh_n.T).astype(BF16),         # [H, S]
        "prevh_n": prevh_n.astype(BF16),                                 # [S, H]
        "prevc_n": prevc_n.astype(BF16),                                 # [S, H]
        "relcol": np.ascontiguousarray(rel.reshape(S // 128, 128).T),    # [128, S/128]
    }


def _prep_shared(inputs):
    Wc, bc = np.asarray(inputs["Wc"]), np.asarray(inputs["bc"])
    Wwf, bwf = np.asarray(inputs["Wwf"]), np.asarray(inputs["bwf"])
    Wuf, buf = np.asarray(inputs["Wuf"]), np.asarray(inputs["buf"])
    return {
        "Wwf_T": np.ascontiguousarray(Wwf.T).astype(BF16),               # [D, H]
        "Wuf_T": np.ascontiguousarray(Wuf.T).astype(BF16),               # [H, H]
        "Wc_T": np.ascontiguousarray(Wc.T).astype(BF16),                 # [D+H, 3H]
        "ones_r": np.ones((1, 128), BF16),                               # [1, 128]
        "b1_r": (bwf + buf)[None, :].astype(BF16),                       # [1, H]
        "bc_r": bc[None, :].astype(BF16),                                # [1, 3H]
        "iota": np.broadcast_to(
            np.arange(128, dtype=np.float32)[None, :], (128, 128)
        ).copy(),                                                        # [128, 128]
        "ident": np.eye(128, dtype=np.float32).astype(BF16),             # [128, 128]
    }


# ---------------------------------------------------------------------------
# Device program (identical for all cores; per-core data differs)
# ---------------------------------------------------------------------------
def _build_program(kmax, repeat=1):
    """repeat>1 wraps the whole body in a hardware loop (timing harness only)."""
    SLOTS = NW * kmax
    S = SLOTS * 128

    nc = bacc.Bacc(None, target_bir_lowering=False)
    d_xT = nc.dram_tensor("x_T", [D, NL], BF, kind="ExternalInput")
    d_phT = nc.dram_tensor("prevh_T", [H, S], BF, kind="ExternalInput")
    d_phn = nc.dram_tensor("prevh_n", [S, H], BF, kind="ExternalInput")
    d_pc = nc.dram_tensor("prevc_n", [S, H], BF, kind="ExternalInput")
    d_rel = nc.dram_tensor("relcol", [128, SLOTS], F32, kind="ExternalInput")
    d_wwf = nc.dram_tensor("Wwf_T", [D, H], BF, kind="ExternalInput")
    d_wuf = nc.dram_tensor("Wuf_T", [H, H], BF, kind="ExternalInput")
    d_wc = nc.dram_tensor("Wc_T", [D + H, H3], BF, kind="ExternalInput")
    d_ones = nc.dram_tensor("ones_r", [1, 128], BF, kind="ExternalInput")
    d_b1 = nc.dram_tensor("b1_r", [1, H], BF, kind="ExternalInput")
    d_bc = nc.dram_tensor("bc_r", [1, H3], BF, kind="ExternalInput")
    d_iota = nc.dram_tensor("iota", [128, 128], F32, kind="ExternalInput")
    d_ident = nc.dram_tensor("ident", [128, 128], BF, kind="ExternalInput")
    d_c = nc.dram_tensor("c_out", [NL, H], F32, kind="ExternalOutput")
    d_h = nc.dram_tensor("h_out", [NL, H], F32, kind="ExternalOutput")

    import contextlib

    with tile.TileContext(nc) as tc:
        with (
            tc.tile_pool(name="const", bufs=1) as cpool,
            tc.tile_pool(name="stream", bufs=3) as spool,
            tc.tile_pool(name="onehot", bufs=2) as opool,
            tc.tile_pool(name="work", bufs=3) as wpool,
            tc.tile_pool(name="gates", bufs=2) as gpool,
            tc.tile_pool(name="pfhg", bufs=2, space="PSUM") as pfhg,
            tc.tile_pool(name="pstp", bufs=1, space="PSUM") as pstp,
            tc.tile_pool(name="pfc", bufs=2, space="PSUM") as pfc,
            tc.tile_pool(name="phtT", bufs=1, space="PSUM") as phtT,
            tc.tile_pool(name="pbig", bufs=2, space="PSUM") as pbig,
            tc.For_i(0, repeat, 1) if repeat > 1 else contextlib.nullcontext(),
        ):
            # ---- resident constants -------------------------------------
            # Small tensors + wwf/xT first (PE's first work is f_inp w0);
            # wc is only needed at window 0's tail, so it loads last.
            iota = cpool.tile([128, 128], F32)
            nc.sync.dma_start(iota[:], d_iota[:])
            ident = cpool.tile([128, 128], BF)
            nc.scalar.dma_start(ident[:], d_ident[:])
            relc = cpool.tile([128, SLOTS], F32)
            nc.sync.dma_start(relc[:], d_rel[:])
            ones = cpool.tile([1, 128], BF)
            nc.sync.dma_start(ones[:], d_ones[:])
            b1r = cpool.tile([1, H], BF)
            nc.sync.dma_start(b1r[:], d_b1[:])
            bcr = cpool.tile([1, H3], BF)
            nc.sync.dma_start(bcr[:], d_bc[:])
            wwf = cpool.tile([128, 4, H], BF)
            nc.sync.dma_start(wwf[:], d_wwf.rearrange("(q p) h -> p q h", p=128))
            wuf = cpool.tile([128, 4, H], BF)
            nc.scalar.dma_start(wuf[:], d_wuf.rearrange("(q p) h -> p q h", p=128))
            xT = cpool.tile([128, 4, NL], BF)
            nc.sync.dma_start(xT[:], d_xT.rearrange("(q p) j -> p q j", p=128))
            wc = cpool.tile([128, 8, H3], BF)

            phT_r = d_phT.rearrange("(q p) (w s) -> p q w s", p=128, w=NW)
            phn_r = d_phn.rearrange("(w k p) h -> p w k h", p=128, w=NW)
            pc_r = d_pc.rearrange("(w k p) h -> p w k h", p=128, w=NW)

            def emit_big(w, hts, fcp):
                """big matmul + gates + outputs for window w."""
                wsl = slice(128 * w, 128 * (w + 1))
                zt = []
                for zc in range(3):
                    zsl = slice(H * zc, H * (zc + 1))
                    bp = pbig.tile([128, H], F32, tag="big")
                    nc.tensor.matmul(bp[:], ones[:], bcr[:, zsl],
                                     start=True, stop=False)
                    for q in range(4):
                        nc.tensor.matmul(bp[:], xT[:, q, wsl], wc[:, q, zsl],
                                         start=False, stop=False)
                    for q in range(4):
                        nc.tensor.matmul(
                            bp[:], hts[:, 128 * q : 128 * (q + 1)],
                            wc[:, 4 + q, zsl], start=False, stop=(q == 3),
                        )
                    zs = gpool.tile([128, H], BF, tag=f"z{zc}")
                    nc.scalar.activation(zs[:], bp[:],
                                         AF.Tanh if zc == 2 else AF.Sigmoid)
                    zt.append(zs)
                zi, zo, zu = zt
                ct = gpool.tile([128, H], F32, tag="ct")
                nc.vector.tensor_tensor(ct[:], zi[:], zu[:], op=ALU.mult)
                nc.vector.tensor_tensor(ct[:], ct[:], fcp[:], op=ALU.add)
                tct = gpool.tile([128, H], BF, tag="tct")
                nc.scalar.activation(tct[:], ct[:], AF.Tanh)
                ht = gpool.tile([128, H], F32, tag="ht")
                nc.vector.tensor_tensor(ht[:], zo[:], tct[:], op=ALU.mult)
                nc.gpsimd.dma_start(d_c[wsl, :], ct[:])
                nc.gpsimd.dma_start(d_h[wsl, :], ht[:])

            prev = None  # (hts, fcp) of previous window
            for w in range(NW):
                wsl = slice(128 * w, 128 * (w + 1))
                # window streams (alternate queues to balance bytes)
                phT = spool.tile([128, 4, kmax * 128], BF, tag="phT")
                nc.sync.dma_start(phT[:], phT_r[:, :, w, :])
                phn = spool.tile([128, kmax, H], BF, tag="phn")
                nc.scalar.dma_start(phn[:], phn_r[:, w, :, :])
                pc = spool.tile([128, kmax, H], BF, tag="pc")
                (nc.sync if w % 2 else nc.scalar).dma_start(pc[:], pc_r[:, w, :, :])
                if w == 0:
                    # wc needed first at window 0's tail (during window 1):
                    # split across both queues, after w0/w1 stream loads.
                    wc_r = d_wc.rearrange("(q p) z -> p q z", p=128)
                    nc.sync.dma_start(wc[:, 0:4, :], wc_r[:, 0:4, :])
                    nc.scalar.dma_start(wc[:, 4:8, :], wc_r[:, 4:8, :])

                # one-hot masks + transposes
                s16t = opool.tile([128, kmax * 128], BF, tag="s16")
                stp = pstp.tile([128, kmax * 128], BF, tag="stp")
                for k in range(kmax):
                    s = w * kmax + k
                    ksl = slice(128 * k, 128 * (k + 1))
                    nc.vector.tensor_scalar(
                        s16t[:, ksl], iota[:], relc[:, s : s + 1], None,
                        op0=ALU.is_equal,
                    )
                    nc.tensor.transpose(stp[:, ksl], s16t[:, ksl], ident[:])
                snc = opool.tile([128, kmax * 128], BF, tag="snc")
                nc.vector.tensor_copy(snc[:], stp[:])

                # f_inp for this window (bias folded in via ones-row matmul)
                fp = pfhg.tile([128, H], F32, tag="fhg")
                nc.tensor.matmul(fp[:], ones[:], b1r[:], start=True, stop=False)
                for q in range(4):
                    nc.tensor.matmul(fp[:], xT[:, q, wsl], wwf[:, q, :],
                                     start=False, stop=(q == 3))
                finp = wpool.tile([128, H], BF, tag="finp")
                nc.scalar.copy(finp[:], fp[:])

                # slot loop: fhg -> sigmoid -> t
                ts = []
                for k in range(kmax):
                    ksl = slice(128 * k, 128 * (k + 1))
                    fhg = pfhg.tile([128, H], F32, tag="fhg")
                    for q in range(4):
                        nc.tensor.matmul(
                            fhg[:], phT[:, q, ksl], wuf[:, q, :],
                            start=(q == 0), stop=False,
                        )
                    nc.tensor.matmul(fhg[:], snc[:, ksl], finp[:],
                                     start=False, stop=True)
                    fjk = wpool.tile([128, H], BF, tag="fjk")
                    nc.scalar.activation(fjk[:], fhg[:], AF.Sigmoid)
                    t = wpool.tile([128, H], BF, tag="t")
                    nc.vector.tensor_tensor(t[:], fjk[:], pc[:, k, :], op=ALU.mult)
                    ts.append(t)

                # fc accumulation (contiguous PSUM group)
                fcp = pfc.tile([128, H], F32, tag="fc")
                for k in range(kmax):
                    nc.tensor.matmul(
                        fcp[:], s16t[:, 128 * k : 128 * (k + 1)], ts[k][:],
                        start=(k == 0), stop=(k == kmax - 1),
                    )

                # h_tilde^T accumulation (q outer: one group per psum slice)
                htp = phtT.tile([128, H], F32, tag="htT")
                for q in range(4):
                    for k in range(kmax):
                        nc.tensor.matmul(
                            htp[:, 128 * q : 128 * (q + 1)],
                            phn[:, k, 128 * q : 128 * (q + 1)],
                            s16t[:, 128 * k : 128 * (k + 1)],
                            start=(k == 0), stop=(k == kmax - 1),
                        )
                hts = gpool.tile([128, H], BF, tag="hts")
                nc.vector.tensor_copy(hts[:], htp[:])

                if prev is not None:
                    emit_big(w - 1, *prev)
                prev = (hts, fcp)
            emit_big(NW - 1, *prev)

    nc.compile()
    return nc


# ---------------------------------------------------------------------------
# Entry point
# ---------------------------------------------------------------------------
def kernel(**inputs):
    inputs = {k: np.asarray(v) for k, v in inputs.items()}
    seg = inputs["seg_ids"]
    assert seg.shape == (E,) and np.all(np.diff(seg) >= 0)

    order, kmax = _plan(seg)
    shared = _prep_shared(inputs)
    in_maps = []
    for core in range(NCORES):
        m = dict(shared)
        m.update(_prep_core(inputs, core, order, kmax))
        in_maps.append(m)

    nc = _build_program(kmax)
    res = run_bass_kernel_spmd(nc, in_maps, list(range(NCORES)))

    c = np.empty((N, H), np.float32)
    h = np.empty((N, H), np.float32)
    for i in range(NCORES):
        idx = order[i * NL : (i + 1) * NL]
        c[idx] = res.results[i]["c_out"]
        h[idx] = res.results[i]["h_out"]
    return (c, h)


# revision 22
# speedup vs baseline: 1.0660x; 1.0660x over previous
"""ChildSum TreeLSTM cell on 8 Trainium2 NeuronCores (Bass/Tile, SPMD).

Sharding: nodes are re-packed host-side into 128 windows of exactly 128
nodes, balanced so every window has exactly 512 children (kmax=4 slots of
128, zero padding) whenever a balanced packing exists; otherwise windows are
padded to the max slot count.  16 windows per core.  Segment sums become
per-window one-hot matmuls accumulated in PSUM; one-hot masks are built on
device from a shipped rel-index tensor, so a single data-independent program
serves all 8 cores.  Outputs are inverse-permuted on the host.

All matmul operands are bf16 (PSUM accumulates fp32); biases are folded into
the matmuls as K=1 ones-row outer products.  DMAs are spread across the two
HWDGE queues (SP, ACT) plus the SWDGE (gpsimd) queue for output stores.

Per-core device program, per window w (software-pipelined one deep):
  s16t[k]  = one-hot S_cn (DVE iota compare), snc = S_cn.T (PE transpose)
  f_inp    = x_w @ Wwf.T + (bwf+buf)                (PSUM, bias matmul)
  fhg[k]   = prevh_k @ Wuf.T + snc_k @ f_inp        (one PSUM group)
  fjk[k]   = sigmoid(fhg[k])   (ACT) ; t[k] = fjk*prevc_k (DVE, bf16)
  fc       = sum_k S_cn_k.T-matmul(t[k])            (PSUM accum)
  htT      = sum_k prevh_k.T-matmul(S_cn_k)         (PSUM accum, q-outer)
  big(w-1) = [x;h_tilde] @ Wc.T + bc ; gates; c = sig(zi)*tanh(zu)+fc ;
  h        = sig(zo)*tanh(c)
"""

import numpy as np
import ml_dtypes

import concourse.bass as bass
import concourse.bacc as bacc
import concourse.mybir as mybir
from concourse import tile
from concourse.bass_utils import run_bass_kernel_spmd

BF16 = ml_dtypes.bfloat16
FP8 = ml_dtypes.float8_e4m3fn
F32 = mybir.dt.float32
BF = mybir.dt.bfloat16
F8 = mybir.dt.float8e4
WS = 16.0  # fp8 weight pre-scale (descaled in the activation reading PSUM)
DR = mybir.MatmulPerfMode.DoubleRow

N, E, D, H = 16384, 65536, 512, 512
NCORES = 8
NL = N // NCORES            # 2048 local nodes
NW = NL // 128              # 16 windows per core
NWIN = NCORES * NW          # 128 windows total
H3 = 3 * H

AF = mybir.ActivationFunctionType
ALU = mybir.AluOpType

# ---------------------------------------------------------------------------
# Host-side planning: balanced node->window packing
# ---------------------------------------------------------------------------
def _plan(seg):
    """Assign nodes to NWIN windows of exactly 128 nodes, minimizing the max
    children per window.  Returns (order, kmax) where order is the node
    permutation (window-major: order[w*128:(w+1)*128] are window w's nodes)
    and kmax = max over windows of ceil(children/128)."""
    cnt = np.bincount(seg, minlength=N).astype(np.int64)
    target = E // NWIN  # 512

    # snake-deal nodes (sorted by child count desc) into windows
    srt = np.argsort(-cnt, kind="stable")
    rows = srt.reshape(N // NWIN, NWIN).copy()
    rows[1::2] = rows[1::2, ::-1]
    # windows[w] = list of node ids
    win = [list(rows[:, w]) for w in range(NWIN)]
    sums = np.array([int(cnt[w].sum()) for w in win], dtype=np.int64)

    # repair: swap nodes between over- and under-full windows until all
    # windows hit the target exactly (possible in practice for random trees).
    # Index nodes by count per window for O(1) swap lookup.
    from collections import defaultdict

    bycnt = [defaultdict(list) for _ in range(NWIN)]
    for w in range(NWIN):
        for nd in win[w]:
            bycnt[w][int(cnt[nd])].append(nd)

    def swap(wa, ca, wb, cb):
        a = bycnt[wa][ca].pop()
        b = bycnt[wb][cb].pop()
        bycnt[wa][cb].append(b)
        bycnt[wb][ca].append(a)
        sums[wa] += cb - ca
        sums[wb] += ca - cb

    for _ in range(200000):
        hi = int(np.argmax(sums))
        lo = int(np.argmin(sums))
        if sums[hi] == target and sums[lo] == target:
            break
        need = min(sums[hi] - target, target - sums[lo])
        done = False
        for d in range(max(int(need), 1), 0, -1):
            for ca, la in bycnt[hi].items():
                if not la:
                    continue
                cb = ca - d
                if bycnt[lo].get(cb):
                    swap(hi, ca, lo, cb)
                    done = True
                    break
            if done:
                break
        if not done:
            break  # no improving swap found

    order = np.empty(N, dtype=np.int64)
    for w in range(NWIN):
        nodes = [nd for la in bycnt[w].values() for nd in la]
        assert len(nodes) == 128
        order[w * 128 : (w + 1) * 128] = nodes
    kmax = int(np.max((sums + 127) // 128))
    return order, kmax


def _ragged(starts, counts):
    total = int(counts.sum())
    if total == 0:
        return np.empty(0, np.int64)
    out = np.ones(total, np.int64)
    nz = counts > 0
    s, c = starts[nz], counts[nz]
    out[0] = s[0]
    idx = np.cumsum(c)[:-1]
    out[idx] = s[1:] - (s[:-1] + c[:-1] - 1)
    return np.cumsum(out)


def _prep_core(inputs, core, order, kmax):
    seg = np.asarray(inputs["seg_ids"])
    x = np.asarray(inputs["x"])
    prev_c, prev_h = np.asarray(inputs["prev_c"]), np.asarray(inputs["prev_h"])
    cnt = np.bincount(seg, minlength=N).astype(np.int64)
    starts = np.concatenate([[0], np.cumsum(cnt)[:-1]])

    S = NW * kmax * 128
    nodes = order[core * NL : (core + 1) * NL]  # window-major node ids

    prevh_n = np.zeros((S, H), np.float32)
    prevc_n = np.zeros((S, H), np.float32)
    rel = np.full((S,), -1.0, np.float32)
    for w in range(NW):
        wn = nodes[w * 128 : (w + 1) * 128]
        idx = _ragged(starts[wn], cnt[wn])
        m = len(idx)
        base = w * kmax * 128
        assert m <= kmax * 128
        prevh_n[base : base + m] = prev_h[idx]
        prevc_n[base : base + m] = prev_c[idx]
        rel[base : base + m] = np.repeat(np.arange(128), cnt[wn]).astype(np.float32)[
            : m
        ]

    xw = x[nodes]
    return {
        "x_T": np.ascontiguousarray(xw.T).astype(BF16),                  # [D, NL]
        "x_T8": np.ascontiguousarray(xw.T).astype(FP8),                  # [D, NL]
        "prevh_T8": np.ascontiguousarray(prevh_n.T).astype(FP8),         # [H, S]
        "prevh_n": prevh_n.astype(BF16),                                 # [S, H]
        "prevc_n": prevc_n.astype(BF16),                                 # [S, H]
        "relcol": np.ascontiguousarray(rel.reshape(S // 128, 128).T),    # [128, S/128]
    }


def _prep_shared(inputs):
    Wc, bc = np.asarray(inputs["Wc"]), np.asarray(inputs["bc"])
    Wwf, bwf = np.asarray(inputs["Wwf"]), np.asarray(inputs["bwf"])
    Wuf, buf = np.asarray(inputs["Wuf"]), np.asarray(inputs["buf"])
    WcT = np.ascontiguousarray(Wc.T)                                     # [D+H, 3H]
    # zi and zu Wc columns in fp8 (x WS), zo columns in bf16
    Wc_z8 = np.concatenate([WcT[:, 0:H], WcT[:, 2 * H : 3 * H]], axis=1)
    return {
        "Wwf_T": np.ascontiguousarray(Wwf.T).astype(BF16),               # [D, H]
        "Wuf_T8": (np.ascontiguousarray(Wuf.T) * WS).astype(FP8),        # [H, H]
        "Wc_zo": np.ascontiguousarray(WcT[:, H : 2 * H]).astype(BF16),   # [D+H, H]
        "Wc_z8": np.ascontiguousarray(Wc_z8 * WS).astype(FP8),           # [D+H, 2H]
        "ones_r": np.ones((1, 128), BF16),                               # [1, 128]
        "b1_r": (bwf + buf)[None, :].astype(BF16),                       # [1, H]
        "bc_zo": bc[None, H : 2 * H].astype(BF16),                       # [1, H]
        "bc16_r": (np.concatenate([bc[0:H], bc[2 * H : 3 * H]]) * WS)[
            None, :
        ].astype(BF16),                                                  # [1, 2H]
        "iota": np.broadcast_to(
            np.arange(128, dtype=np.float32)[None, :], (128, 128)
        ).copy(),                                                        # [128, 128]
        "ident": np.eye(128, dtype=np.float32).astype(BF16),             # [128, 128]
    }


# ---------------------------------------------------------------------------
# Device program (identical for all cores; per-core data differs)
# ---------------------------------------------------------------------------
def _build_program(kmax, repeat=1, mode="full"):
    """repeat>1 wraps the whole body in a hardware loop (timing harness only).
    mode: "full", or probe builds that keep the instruction stream but shrink
    one engine class's work to ~zero: "nodma" (tiny DMAs), "nope" (tiny
    matmuls), "novec" (tiny ACT/DVE ops)."""
    SLOTS = NW * kmax
    S = SLOTS * 128
    DMA = mode != "nodma"
    PE = mode != "nope"
    VEC = mode != "novec"

    nc = bacc.Bacc(None, target_bir_lowering=False)
    d_xT = nc.dram_tensor("x_T", [D, NL], BF, kind="ExternalInput")
    d_xT8 = nc.dram_tensor("x_T8", [D, NL], F8, kind="ExternalInput")
    d_phT8 = nc.dram_tensor("prevh_T8", [H, S], F8, kind="ExternalInput")
    d_phn = nc.dram_tensor("prevh_n", [S, H], BF, kind="ExternalInput")
    d_pc = nc.dram_tensor("prevc_n", [S, H], BF, kind="ExternalInput")
    d_rel = nc.dram_tensor("relcol", [128, SLOTS], F32, kind="ExternalInput")
    d_wwf = nc.dram_tensor("Wwf_T", [D, H], BF, kind="ExternalInput")
    d_wuf8 = nc.dram_tensor("Wuf_T8", [H, H], F8, kind="ExternalInput")
    d_wzo = nc.dram_tensor("Wc_zo", [D + H, H], BF, kind="ExternalInput")
    d_wz8 = nc.dram_tensor("Wc_z8", [D + H, 2 * H], F8, kind="ExternalInput")
    d_ones = nc.dram_tensor("ones_r", [1, 128], BF, kind="ExternalInput")
    d_b1 = nc.dram_tensor("b1_r", [1, H], BF, kind="ExternalInput")
    d_bczo = nc.dram_tensor("bc_zo", [1, H], BF, kind="ExternalInput")
    d_bc16 = nc.dram_tensor("bc16_r", [1, 2 * H], BF, kind="ExternalInput")
    d_iota = nc.dram_tensor("iota", [128, 128], F32, kind="ExternalInput")
    d_ident = nc.dram_tensor("ident", [128, 128], BF, kind="ExternalInput")
    d_c = nc.dram_tensor("c_out", [NL, H], F32, kind="ExternalOutput")
    d_h = nc.dram_tensor("h_out", [NL, H], F32, kind="ExternalOutput")

    import contextlib

    # Probe modes keep the full instruction stream and dependency graph but
    # shrink one engine class's work to near-zero via tiny access patterns.
    def _tiny(ap, n=32):
        idx = tuple(slice(0, 1) for _ in range(ap.ndim - 1)) + (slice(0, n),)
        return ap[idx]

    def _dma(eng, out, in_, **k):
        if DMA:
            eng.dma_start(out, in_, **k)
        else:
            eng.dma_start(_tiny(out), _tiny(in_), **k)

    def _mm(out, lhsT, rhs, **k):
        if PE:
            nc.tensor.matmul(out, lhsT, rhs, **k)
        elif rhs.ndim == 3:  # DoubleRow probe: rhs [K, 2, N] -> out free N/...
            nc.tensor.matmul(out[:, :4], lhsT, rhs[:, :, :4], **k)
        else:
            nc.tensor.matmul(out[:, :8], lhsT, rhs[:, :8], **k)

    def _tr(out, in_, ident_):
        if PE:
            nc.tensor.transpose(out, in_, ident_)
        else:
            nc.tensor.matmul(out[:, :8], in_, ident_[:, :8], is_transpose=True)

    def _sm(ap, n=8):
        idx = (slice(None),) + tuple(slice(0, 1) for _ in range(ap.ndim - 2))
        return ap[idx + (slice(0, n),)]

    def _act(out, in_, func, **k):
        if VEC:
            nc.scalar.activation(out, in_, func, **k)
        else:
            nc.scalar.activation(_sm(out), _sm(in_), func, **k)

    def _cp(eng, out, in_):
        if VEC:
            eng(out, in_)
        else:
            eng(_sm(out), _sm(in_))

    def _tt(out, a, b, op):
        if VEC:
            nc.vector.tensor_tensor(out, a, b, op=op)
        else:
            nc.vector.tensor_tensor(_sm(out), _sm(a), _sm(b), op=op)

    def _ts(out, in0, sc, x, op0):
        if VEC:
            nc.vector.tensor_scalar(out, in0, sc, x, op0=op0)
        else:
            nc.vector.tensor_scalar(_sm(out), _sm(in0), sc, x, op0=op0)

    with tile.TileContext(nc) as tc:
        with (
            tc.tile_pool(name="const", bufs=2) as cpool,
            tc.tile_pool(name="stream", bufs=3) as spool,
            tc.tile_pool(name="onehot", bufs=2) as opool,
            tc.tile_pool(name="work", bufs=3) as wpool,
            tc.tile_pool(name="gates", bufs=2) as gpool,
            tc.tile_pool(name="pfhg", bufs=2, space="PSUM") as pfhg,
            tc.tile_pool(name="pstp", bufs=1, space="PSUM") as pstp,
            tc.tile_pool(name="pfc", bufs=2, space="PSUM") as pfc,
            tc.tile_pool(name="phtT", bufs=1, space="PSUM") as phtT,
            tc.tile_pool(name="pbig", bufs=2, space="PSUM") as pbig,
            tc.For_i(0, repeat, 1) if repeat > 1 else contextlib.nullcontext(),
        ):
            # ---- resident constants -------------------------------------
            # Small tensors + wwf/xT first (PE's first work is f_inp w0);
            # wc is only needed at window 0's tail, so it loads last.
            iota = cpool.tile([128, 128], F32)
            _dma(nc.sync, iota[:], d_iota[:])
            ident = cpool.tile([128, 128], BF)
            _dma(nc.scalar, ident[:], d_ident[:])
            relc = cpool.tile([128, SLOTS], F32)
            _dma(nc.sync, relc[:], d_rel[:])
            ones = cpool.tile([1, 128], BF)
            _dma(nc.sync, ones[:], d_ones[:])
            b1r = cpool.tile([1, H], BF)
            _dma(nc.sync, b1r[:], d_b1[:])
            bczo = cpool.tile([1, H], BF)
            _dma(nc.sync, bczo[:], d_bczo[:])
            bc16 = cpool.tile([1, 2 * H], BF)
            _dma(nc.sync, bc16[:], d_bc16[:])
            wwf = cpool.tile([128, 4, H], BF)
            _dma(nc.sync, wwf[:], d_wwf.rearrange("(q p) h -> p q h", p=128))
            wuf8 = cpool.tile([128, 4, H], F8)
            _dma(nc.scalar, wuf8[:], d_wuf8.rearrange("(q p) h -> p q h", p=128))
            xT = cpool.tile([128, 4, NL], BF)
            xT_r = d_xT.rearrange("(q p) j -> p q j", p=128)
            # window-0 slice first so f_inp(0) starts ~5us earlier
            _dma(nc.sync, xT[:, :, 0:128], xT_r[:, :, 0:128])
            _dma(nc.sync, xT[:, :, 128:NL], xT_r[:, :, 128:NL])
            xT8 = cpool.tile([128, 4, NL], F8)
            _dma(nc.scalar, xT8[:], d_xT8.rearrange("(q p) j -> p q j", p=128))
            wzo = cpool.tile([128, 8, H], BF)
            wz8 = cpool.tile([128, 8, 2 * H], F8)

            phT_r = d_phT8.rearrange("(q p) (w s) -> p q w s", p=128, w=NW)
            phn_r = d_phn.rearrange("(w k p) h -> p w k h", p=128, w=NW)
            pc_r = d_pc.rearrange("(w k p) h -> p w k h", p=128, w=NW)

            def emit_big(w, hts, hts8, fcp):
                """big matmul + gates + outputs for window w.  zi and zu run
                as fp8 DoubleRow (weights pre-scaled by WS, descaled in the
                activation); zo runs bf16 for precision."""
                wsl = slice(128 * w, 128 * (w + 1))
                zt = []
                for zc, zsl in (("i", slice(0, H)), ("u", slice(H, 2 * H))):
                    bp = pbig.tile([128, H], F32, tag="big")
                    _mm(bp[:], ones[:], bc16[:, zsl], start=True, stop=False)
                    for t in range(2):
                        _mm(bp[:], xT8[:, 2 * t : 2 * t + 2, wsl],
                            wz8[:, 2 * t : 2 * t + 2, zsl],
                            start=False, stop=False, perf_mode=DR)
                    for t in range(2):
                        _mm(bp[:], hts8[:, 2 * t : 2 * t + 2, :],
                            wz8[:, 4 + 2 * t : 6 + 2 * t, zsl],
                            start=False, stop=(t == 1), perf_mode=DR)
                    zs = gpool.tile([128, H], BF, tag=f"z{zc}")
                    _act(zs[:], bp[:], AF.Tanh if zc == "u" else AF.Sigmoid,
                         scale=1.0 / WS)
                    zt.append(zs)
                zi, zu = zt
                bp = pbig.tile([128, H], F32, tag="big")
                _mm(bp[:], ones[:], bczo[:], start=True, stop=False)
                for q in range(4):
                    _mm(bp[:], xT[:, q, wsl], wzo[:, q, :],
                        start=False, stop=False)
                for q in range(4):
                    _mm(bp[:], hts[:, q, :], wzo[:, 4 + q, :],
                        start=False, stop=(q == 3))
                zo = gpool.tile([128, H], BF, tag="zo")
                _act(zo[:], bp[:], AF.Sigmoid)
                ct = gpool.tile([128, H], F32, tag="ct")
                _tt(ct[:], zi[:], zu[:], op=ALU.mult)
                _tt(ct[:], ct[:], fcp[:], op=ALU.add)
                tct = gpool.tile([128, H], BF, tag="tct")
                _act(tct[:], ct[:], AF.Tanh)
                ht = gpool.tile([128, H], F32, tag="ht")
                _tt(ht[:], zo[:], tct[:], op=ALU.mult)
                _dma(nc.gpsimd, d_c[wsl, :], ct[:])
                _dma(nc.gpsimd, d_h[wsl, :], ht[:])

            prev = None  # (hts, fcp) of previous window
            for w in range(NW):
                wsl = slice(128 * w, 128 * (w + 1))
                # window streams (alternate queues to balance bytes)
                phT = spool.tile([128, 4, kmax * 128], F8, tag="phT")
                _dma(nc.sync, phT[:], phT_r[:, :, w, :])
                phn = spool.tile([128, kmax, H], BF, tag="phn")
                _dma(nc.scalar, phn[:], phn_r[:, w, :, :])
                pc = spool.tile([128, kmax, H], BF, tag="pc")
                _dma(nc.sync if w % 2 else nc.scalar, pc[:], pc_r[:, w, :, :])
                if w == 0:
                    # Wc needed first at window 0's tail (during window 1):
                    # split across both queues, after w0/w1 stream loads.
                    _dma(nc.sync, wz8[:], d_wz8.rearrange("(q p) z -> p q z", p=128))
                    _dma(nc.scalar, wzo[:], d_wzo.rearrange("(q p) z -> p q z", p=128))

                # one-hot masks + transposes
                s16t = opool.tile([128, kmax * 128], BF, tag="s16")
                stp = pstp.tile([128, kmax * 128], BF, tag="stp")
                for k in range(kmax):
                    s = w * kmax + k
                    ksl = slice(128 * k, 128 * (k + 1))
                    _ts(s16t[:, ksl], iota[:], relc[:, s : s + 1], None,
                        op0=ALU.is_equal)
                    _tr(stp[:, ksl], s16t[:, ksl], ident[:])
                snc = opool.tile([128, kmax * 128], BF, tag="snc")
                _cp(nc.vector.tensor_copy, snc[:], stp[:])

                # f_inp for this window (bias folded in via ones-row matmul);
                # scaled by WS on copy-out so it can join the WS-scaled fp8
                # fh accumulation group.
                fp = pfhg.tile([128, H], F32, tag="fhg")
                _mm(fp[:], ones[:], b1r[:], start=True, stop=False)
                for q in range(4):
                    _mm(fp[:], xT[:, q, wsl], wwf[:, q, :],
                        start=False, stop=(q == 3))
                finp = wpool.tile([128, H], BF, tag="finp")
                _act(finp[:], fp[:], AF.Identity, scale=WS)

                # slot loop: fhg (fp8 DoubleRow fh + bf16 gather) -> sigmoid -> t
                ts = []
                for k in range(kmax):
                    ksl = slice(128 * k, 128 * (k + 1))
                    fhg = pfhg.tile([128, H], F32, tag="fhg")
                    for t2 in range(2):
                        _mm(fhg[:], phT[:, 2 * t2 : 2 * t2 + 2, ksl],
                            wuf8[:, 2 * t2 : 2 * t2 + 2, :],
                            start=(t2 == 0), stop=False, perf_mode=DR)
                    _mm(fhg[:], snc[:, ksl], finp[:], start=False, stop=True)
                    fjk = wpool.tile([128, H], BF, tag="fjk")
                    _act(fjk[:], fhg[:], AF.Sigmoid, scale=1.0 / WS)
                    t = wpool.tile([128, H], BF, tag="t")
                    _tt(t[:], fjk[:], pc[:, k, :], op=ALU.mult)
                    ts.append(t)

                # fc accumulation (contiguous PSUM group)
                fcp = pfc.tile([128, H], F32, tag="fc")
                for k in range(kmax):
                    _mm(fcp[:], s16t[:, 128 * k : 128 * (k + 1)], ts[k][:],
                        start=(k == 0), stop=(k == kmax - 1))

                # h_tilde^T accumulation (q outer: one group per psum slice)
                htp = phtT.tile([128, 4, 128], F32, tag="htT")
                for q in range(4):
                    for k in range(kmax):
                        _mm(htp[:, q, :],
                            phn[:, k, 128 * q : 128 * (q + 1)],
                            s16t[:, 128 * k : 128 * (k + 1)],
                            start=(k == 0), stop=(k == kmax - 1))
                hts = gpool.tile([128, 4, 128], BF, tag="hts")
                _cp(nc.vector.tensor_copy, hts[:], htp[:])
                hts8 = gpool.tile([128, 4, 128], F8, tag="hts8")
                _cp(nc.vector.tensor_copy, hts8[:], htp[:])

                if prev is not None:
                    emit_big(w - 1, *prev)
                prev = (hts, hts8, fcp)
            emit_big(NW - 1, *prev)

    nc.compile()
    return nc


# ---------------------------------------------------------------------------
# Entry point
# ---------------------------------------------------------------------------
def kernel(**inputs):
    inputs = {k: np.asarray(v) for k, v in inputs.items()}
    seg = inputs["seg_ids"]
    assert seg.shape == (E,) and np.all(np.diff(seg) >= 0)

    order, kmax = _plan(seg)
    shared = _prep_shared(inputs)
    in_maps = []
    for core in range(NCORES):
        m = dict(shared)
        m.update(_prep_core(inputs, core, order, kmax))
        in_maps.append(m)

    nc = _build_program(kmax)
    res = run_bass_kernel_spmd(nc, in_maps, list(range(NCORES)))

    c = np.empty((N, H), np.float32)
    h = np.empty((N, H), np.float32)
    for i in range(NCORES):
        idx = order[i * NL : (i + 1) * NL]
        c[idx] = res.results[i]["c_out"]
        h[idx] = res.results[i]["h_out"]
    return (c, h)
